# revision 6
# baseline (speedup 1.0000x reference)
"""Trainium2 Bass kernel for nn_AAFM (sparse attention with distance decay).

Math (per batch b):
    q = query @ Wq.T + bq ; k = key @ Wk.T + bk ; v = value @ Wv.T + bv
    exp_A = exp(-alpha*log2(N)*dist)            [n_q, n_k]
    num   = exp_A @ (exp(k) * v)                [n_q, d]
    den   = exp_A @ exp(k) + 1e-8               [n_q, d]
    out   = sigmoid(q) * num / den

Transforms used here:
  - bk cancels exactly in num/den (columnwise positive factor exp(bk)), so it
    is dropped (the 1e-8 placement differs by a negligible ~1e-10 relative).
  - sigmoid(q) = 0.5*(tanh(q/2)+1): tanh shares the ACT "exp_and_others"
    table set with exp, avoiding per-batch ~2.7us table reloads. The 0.5 is
    folded into Wv/bv on the host, so out = (tanh(q/2)+1) * num' / den.
  - All device DMAs are natural-layout: the host pre-transposes dist to
    [b, k, q] and q/k/v to [b, d, n] and pre-transposes the weights, so the
    TensorEngine contraction dim is always the SBUF partition dim.
  - Projections run as float32r matmuls straight from f32 DMA loads (no
    cast); the big exp_A @ [exp_K*v | exp_K] matmul runs in bf16.
  - Data-parallel over batch: 32 batches -> 8 cores x 4, no collectives.
"""

import sys

for _p in ("/opt/trn_rl_repo",):
    if _p not in sys.path:
        sys.path.append(_p)

import numpy as np

import concourse.bass as bass
import concourse.tile as tile
from concourse import bacc, mybir
from concourse.bass_utils import run_bass_kernel_spmd

N_CORES = 8
B = 32
N = 1024  # graph size
D = 256  # d_model
BPC = B // N_CORES  # batches per core
KT = N // 128  # 8 row tiles of 128
F32 = mybir.dt.float32
F32R = mybir.dt.float32r
BF16 = mybir.dt.bfloat16
Alu = mybir.AluOpType
Act = mybir.ActivationFunctionType


def build_graph(c_coef: float):
    """Build the SPMD single-core graph (same on all 8 cores)."""
    nc = bacc.Bacc(
        "TRN2", target_bir_lowering=False, debug=False, num_devices=N_CORES
    )

    distT = nc.declare_dram_parameter("distT", [BPC, N, N], F32, isOutput=False)
    qT = nc.declare_dram_parameter("qT", [BPC, D, N], F32, isOutput=False)
    kT = nc.declare_dram_parameter("kT", [BPC, D, N], F32, isOutput=False)
    vT = nc.declare_dram_parameter("vT", [BPC, D, N], F32, isOutput=False)
    WqT = nc.declare_dram_parameter("WqT", [D, D], F32, isOutput=False)
    WkT = nc.declare_dram_parameter("WkT", [D, D], F32, isOutput=False)
    WvT = nc.declare_dram_parameter("WvT", [D, D], F32, isOutput=False)
    bq_d = nc.declare_dram_parameter("bq", [1, D], F32, isOutput=False)
    bv_d = nc.declare_dram_parameter("bv", [1, D], F32, isOutput=False)
    ones_d = nc.declare_dram_parameter("ones", [1, 128], F32, isOutput=False)
    out_d = nc.declare_dram_parameter("out", [BPC, N, D], F32, isOutput=True)

    def r32(ap):
        return ap.bitcast(F32R)

    with tile.TileContext(nc) as tc:
        with (
            tc.tile_pool(name="const", bufs=1) as const_p,
            tc.tile_pool(name="qkv", bufs=2) as qkv_p,
            tc.tile_pool(name="dist", bufs=2) as dist_p,
            tc.tile_pool(name="expA", bufs=2) as expA_p,
            tc.tile_pool(name="ekv", bufs=2) as ekv_p,
            tc.tile_pool(name="tanh", bufs=2) as tanh_p,
            tc.tile_pool(name="eps", bufs=3) as eps_p,
            tc.tile_pool(name="outst", bufs=2) as out_p,
            tc.tile_pool(name="ppsum", bufs=2, space="PSUM") as ppsum,
            tc.tile_pool(name="mpsum", bufs=2, space="PSUM") as mpsum,
        ):
            # ---- constants (once) ----
            w_tiles = {}
            for nm, wd in (("wk", WkT), ("wv", WvT), ("wq", WqT)):
                wt = const_p.tile([128, 2, D], F32R, tag=nm)
                nc.sync.dma_start(
                    wt[:], wd[:].rearrange("(j p) e -> p j e", p=128).bitcast(F32R)
                )
                w_tiles[nm] = wt
            bq_t = const_p.tile([1, D], F32R, tag="bq")
            nc.sync.dma_start(bq_t[:], bq_d[:].bitcast(F32R))
            bv_t = const_p.tile([1, D], F32R, tag="bv")
            nc.sync.dma_start(bv_t[:], bv_d[:].bitcast(F32R))
            ones_t = const_p.tile([1, 128], F32R, tag="ones")
            nc.sync.dma_start(ones_t[:], ones_d[:].bitcast(F32R))

            for bi in range(BPC):
                # ---- input DMAs (kT first: k-projection unblocks first) ----
                xt = {}
                for nm, xd in (("kT", kT), ("vT", vT), ("qT", qT)):
                    t = qkv_p.tile([128, 2, N], F32R, tag=nm)
                    nc.sync.dma_start(
                        t[:], xd[bi].rearrange("(j p) n -> p j n", p=128).bitcast(F32R)
                    )
                    xt[nm] = t

                expA = expA_p.tile([128, KT, N], BF16, tag="expA")
                dists = []
                for c in range(2):
                    dt_t = dist_p.tile([128, 4, N], F32, tag="dist")
                    nc.sync.dma_start(
                        dt_t[:],
                        distT[bi, c * 512 : (c + 1) * 512, :].rearrange(
                            "(j p) q -> p j q", p=128
                        ),
                    )
                    dists.append(dt_t)

                # ---- exp_A^T = exp(-c * distT)  (bf16) ----
                for c in range(2):
                    nc.scalar.activation(
                        expA[:, c * 4 : (c + 1) * 4, :],
                        dists[c][:],
                        Act.Exp,
                        scale=-c_coef,
                    )

                # ---- projections (pairs of row tiles), float32r ----
                ekv = ekv_p.tile([128, KT, 2 * D], BF16, tag="ekv")
                tanh_t = tanh_p.tile([128, KT, D], BF16, tag="tanh")
                for g in range(KT // 2):
                    t0 = 2 * g
                    # k-projection pair -> exp -> ekv[..., D:2D]
                    kp = ppsum.tile([128, 2, D], F32, tag="kp")
                    for j in range(2):
                        for dt in range(2):
                            nc.tensor.matmul(
                                kp[:, j, :],
                                xt["kT"][:, dt, (t0 + j) * 128 : (t0 + j + 1) * 128],
                                w_tiles["wk"][:, dt, :],
                                start=(dt == 0),
                                stop=(dt == 1),
                            )
                    nc.scalar.activation(
                        ekv[:, t0 : t0 + 2, D : 2 * D], kp[:], Act.Exp
                    )

                    # v-projection pair (bias via K=1 ones matmul)
                    vp = ppsum.tile([128, 2, D], F32, tag="vp")
                    for j in range(2):
                        for dt in range(2):
                            nc.tensor.matmul(
                                vp[:, j, :],
                                xt["vT"][:, dt, (t0 + j) * 128 : (t0 + j + 1) * 128],
                                w_tiles["wv"][:, dt, :],
                                start=(dt == 0),
                                stop=False,
                            )
                        nc.tensor.matmul(
                            vp[:, j, :], ones_t[:], bv_t[:],
                            start=False, stop=True,
                        )
                    nc.vector.tensor_mul(
                        ekv[:, t0 : t0 + 2, 0:D],
                        ekv[:, t0 : t0 + 2, D : 2 * D],
                        vp[:],
                    )

                    # q-projection pair (with bias) -> tanh(q/2)
                    qp = ppsum.tile([128, 2, D], F32, tag="qp")
                    for j in range(2):
                        for dt in range(2):
                            nc.tensor.matmul(
                                qp[:, j, :],
                                xt["qT"][:, dt, (t0 + j) * 128 : (t0 + j + 1) * 128],
                                w_tiles["wq"][:, dt, :],
                                start=(dt == 0),
                                stop=False,
                            )
                        nc.tensor.matmul(
                            qp[:, j, :], ones_t[:], bq_t[:],
                            start=False, stop=True,
                        )
                    nc.scalar.activation(
                        tanh_t[:, t0 : t0 + 2, :], qp[:], Act.Tanh, scale=0.5
                    )

                # ---- main matmuls + epilogue per q tile ----
                out_t = out_p.tile([128, KT, D], F32, tag="outst")
                for qi in range(KT):
                    mm = mpsum.tile([128, 2 * D], F32, tag="mm")
                    for t in range(KT):
                        nc.tensor.matmul(
                            mm[:],
                            expA[:, t, qi * 128 : (qi + 1) * 128],
                            ekv[:, t, :],
                            start=(t == 0),
                            stop=(t == KT - 1),
                        )
                    # out = (tanh+1) * num' * ~1/den   (0.5 folded into Wv/bv)
                    r = eps_p.tile([128, D], F32, tag="r")
                    nc.vector.reciprocal_approx_fast(r[:], mm[:, D : 2 * D])
                    p = eps_p.tile([128, D], F32, tag="p")
                    nc.vector.scalar_tensor_tensor(
                        p[:], tanh_t[:, qi, :], 1.0, mm[:, 0:D], Alu.add, Alu.mult
                    )
                    nc.gpsimd.tensor_mul(out_t[:, qi, :], p[:], r[:])

                nc.sync.dma_start(
                    out_d[bi].rearrange("(j p) e -> p j e", p=128), out_t[:]
                )

    nc.compile()
    return nc


def prepare_in_maps(inputs: dict):
    query = np.asarray(inputs["query"], dtype=np.float32)
    key_ = np.asarray(inputs["key_"], dtype=np.float32)
    value = np.asarray(inputs["value"], dtype=np.float32)
    dist = np.asarray(inputs["dist"], dtype=np.float32)
    Wq = np.asarray(inputs["Wq"], dtype=np.float32)
    Wk = np.asarray(inputs["Wk"], dtype=np.float32)
    Wv = np.asarray(inputs["Wv"], dtype=np.float32)
    bq = np.asarray(inputs["bq"], dtype=np.float32)
    bv = np.asarray(inputs["bv"], dtype=np.float32)
    alpha_raw = np.asarray(inputs["alpha_raw"], dtype=np.float64)

    alpha = float(np.logaddexp(0.0, alpha_raw)) + 1e-6  # softplus + eps
    c_coef = float(alpha * np.log2(float(N)))

    distT = np.ascontiguousarray(dist.transpose(0, 2, 1))
    qT = np.ascontiguousarray(query.transpose(0, 2, 1))
    kT = np.ascontiguousarray(key_.transpose(0, 2, 1))
    vT = np.ascontiguousarray(value.transpose(0, 2, 1))
    WqT = np.ascontiguousarray(Wq.T)
    WkT = np.ascontiguousarray(Wk.T)
    WvT = np.ascontiguousarray(Wv.T) * 0.5  # fold sigmoid's 0.5
    bq2 = np.ascontiguousarray(bq.reshape(1, D))
    bv2 = np.ascontiguousarray(bv.reshape(1, D)) * 0.5

    in_maps = []
    for i in range(N_CORES):
        s = slice(i * BPC, (i + 1) * BPC)
        in_maps.append(
            {
                "distT": distT[s],
                "qT": qT[s],
                "kT": kT[s],
                "vT": vT[s],
                "WqT": WqT,
                "WkT": WkT,
                "WvT": WvT,
                "bq": bq2,
                "bv": bv2,
                "ones": np.ones((1, 128), dtype=np.float32),
            }
        )
    return in_maps, c_coef


def run_sharded(inputs: dict, trace: bool = False):
    """Returns (full_output [32,1024,256] f32, BassKernelResults)."""
    in_maps, c_coef = prepare_in_maps(inputs)
    nc = build_graph(c_coef)
    res = run_bass_kernel_spmd(
        nc, in_maps, core_ids=list(range(N_CORES)), trace=trace
    )
    out = np.concatenate(
        [res.results[i]["out"] for i in range(N_CORES)], axis=0
    ).astype(np.float32)
    return out, res


def kernel(**inputs) -> np.ndarray:
    out, _ = run_sharded(inputs, trace=False)
    return out
